# revision 10
# baseline (speedup 1.0000x reference)
"""Trainium2 Bass kernel: MultiHeadCrossAttentionWithBias.

Reference computation (per batch b):
  q_u = scale*(u_enc @ wq + wq_b); k/v from e_enc (and vice versa)
  ue_w = softmax(q_u k_e^T + bppw*bpp + bppb + mask*-inf); u_ctx = ue_w @ v_e
  u_update = u_ctx @ wo + wo_b                     (same mirrored for e)

Sharding: 8 fully independent attention units (batch b, direction d).
Core i = (d, b) handles one unit end-to-end; no collectives.

v3 design notes (fp32r baseline 219us -> v2 187us -> this):
 - ALL matmul operands 16-bit: fp32r ran in fp32_mode=HIGH and kept the
   PE HAM-throttled; fp16 (10-bit mantissa) for enc/w/q/k/wo/ctxn keeps
   error near baseline, bf16 for the exp-range tensors (E, EBM, v).
 - Per-(h,kc) bias-add in PSUM replaced by a bf16 multiply with
   EBM[k,q] = exp(bppw*bpp + maskneg): exp(S+CB) = exp(S)*EBM.
   bppb and the reference's +EPS shift all logits uniformly and cancel
   in softmax. Mask encodes as -60000 (fp16) -> exp = 0, which also
   implements the post-softmax re-mask.
 - ACT does exp only; attention pace is 1 exp per (h,kc) (~1us).
 - q/k projection blocks are interleaved between attention head-pairs:
   shortens the serial prologue and re-warms the PE HAM at each pair
   boundary (micro-idles during ACT-paced attention drop PE to 1.2GHz).
 - den reciprocal straight from PSUM (reciprocal_approx_fast), bounced
   through DRAM for the partition broadcast; every 3rd E*EBM multiply
   runs on gpsimd to keep DVE under the exp pace.

Host prep is layout/precision only (transposes, slices, fp16/bf16
rounding, mask -> {0,-60000} encoding); all FLOPs run on device.
"""

import numpy as np
from contextlib import ExitStack

import ml_dtypes

import concourse.bass as bass
import concourse.tile as tile
import concourse.bacc as bacc
import concourse.mybir as mybir
from concourse import bass_utils

F32 = mybir.dt.float32
F16 = mybir.dt.float16
BF16 = mybir.dt.bfloat16
AF = mybir.ActivationFunctionType
ALU = mybir.AluOpType
BF16NP = ml_dtypes.bfloat16

B, L, D, H, HD = 4, 1024, 512, 8, 64
P = 128
FH = H * HD            # 512
SCALE = 1.0 / np.sqrt(HD)
N_CORES = 8
LAGS = (4, 3)          # kc lag between QK and PV streams (h==0, h>0)


def bcast_ap(dram_ap, parts):
    """Partition-step-0 broadcast AP over a DRAM row."""
    return bass.AP(tensor=dram_ap.tensor, offset=dram_ap.offset,
                   ap=[[0, parts]] + list(dram_ap.ap))


def build_module():
    nc = bacc.Bacc("TRN2", target_bir_lowering=False, debug=False)

    encQT_d = nc.dram_tensor("encQT", [D, L], F16, kind="ExternalInput")
    encKT_d = nc.dram_tensor("encKT", [D, L], F16, kind="ExternalInput")
    wq_d = nc.dram_tensor("wq", [D, FH], F16, kind="ExternalInput")
    wk_d = nc.dram_tensor("wk", [D, FH], F16, kind="ExternalInput")
    wv_d = nc.dram_tensor("wv", [D, FH], F16, kind="ExternalInput")
    wo_d = nc.dram_tensor("wo", [FH, D], F16, kind="ExternalInput")
    bpp_d = nc.dram_tensor("bpp", [L, L], F16, kind="ExternalInput")
    mneg_d = nc.dram_tensor("mneg", [L, L], F16, kind="ExternalInput")
    wqb_d = nc.dram_tensor("wqb", [FH], F32, kind="ExternalInput")
    wkb_d = nc.dram_tensor("wkb", [FH], F32, kind="ExternalInput")
    wvb_d = nc.dram_tensor("wvb", [FH], F32, kind="ExternalInput")
    wob_d = nc.dram_tensor("wob", [D], F32, kind="ExternalInput")
    bppw_d = nc.dram_tensor("bppw", [1, 1], F32, kind="ExternalInput")
    out_d = nc.dram_tensor("out", [L, D], F32, kind="ExternalOutput")
    den_d = nc.dram_tensor("den_scratch", [H, L], F32, kind="Internal")

    with tile.TileContext(nc) as tc, ExitStack() as ctx:
        const = ctx.enter_context(tc.tile_pool(name="const", bufs=1))
        qkT_p = ctx.enter_context(tc.tile_pool(name="qkT", bufs=8))
        v_p = ctx.enter_context(tc.tile_pool(name="v", bufs=8))
        wo_p = ctx.enter_context(tc.tile_pool(name="wo", bufs=4))
        ebm_p = ctx.enter_context(tc.tile_pool(name="ebm", bufs=8))
        enc_p = ctx.enter_context(tc.tile_pool(name="enc", bufs=8))
        w_p = ctx.enter_context(tc.tile_pool(name="wqkv", bufs=12))
        cbt_p = ctx.enter_context(tc.tile_pool(name="cbtmp", bufs=4))
        ctxn_p = ctx.enter_context(tc.tile_pool(name="ctxn", bufs=4))
        den_p = ctx.enter_context(tc.tile_pool(name="den", bufs=4))
        er_p = ctx.enter_context(tc.tile_pool(name="er", bufs=3))
        e_p = ctx.enter_context(tc.tile_pool(name="e", bufs=7))
        rb_p = ctx.enter_context(tc.tile_pool(name="rb", bufs=2))
        ps_s = tc.alloc_tile_pool(name="ps_s", bufs=2, space="PSUM")
        ps_c = tc.alloc_tile_pool(name="ps_c", bufs=2, space="PSUM")

        # ---- small bias prep (tiny DMAs) ----
        bw_col = const.tile([P, 1], F32)
        nc.gpsimd.dma_start(bw_col[:], bcast_ap(bppw_d.ap()[0:1, :], P))
        wqb_raw = const.tile([P, 4], F32)
        nc.gpsimd.dma_start(wqb_raw[:], wqb_d.ap().rearrange("(c p) -> p c", p=P))
        wqb_sc = const.tile([P, 4], F32)
        nc.vector.tensor_scalar_mul(wqb_sc[:], wqb_raw[:], float(SCALE))
        wkb_c = const.tile([P, 4], F32)
        nc.gpsimd.dma_start(wkb_c[:], wkb_d.ap().rearrange("(c p) -> p c", p=P))
        wvb_bc = const.tile([P, FH], F32)
        nc.gpsimd.dma_start(wvb_bc[:], bcast_ap(wvb_d.ap(), P))
        wob_bc = const.tile([P, D], F32)
        nc.gpsimd.dma_start(wob_bc[:], bcast_ap(wob_d.ap(), P))

        # ---- input loads ----
        # sync queue: weights + encoders (projection-critical), then the
        # second half of bpp/mneg. gpsimd queue: first half of bpp/mneg
        # (needed early for EBM), then wo.
        eq, ek = [], []
        wq_t, wk_t, wv_t = [], [], []
        for w_dram, wlst, elst, edram in (
            (wq_d, wq_t, eq, encQT_d), (wk_d, wk_t, ek, encKT_d),
            (wv_d, wv_t, None, None),
        ):
            for dc in range(4):
                t = w_p.tile([P, FH], F16, tag="w", name=f"w_{w_dram.name}{dc}")
                nc.sync.dma_start(t[:], w_dram.ap()[dc * P:(dc + 1) * P, :])
                wlst.append(t)
            if elst is None:
                continue
            for dc in range(4):
                t = enc_p.tile([P, L], F16, tag="enc",
                               name=f"enc_{edram.name}{dc}")
                nc.sync.dma_start(t[:], edram.ap()[dc * P:(dc + 1) * P, :])
                elst.append(t)
        bm_tiles = {}
        for kc in range(8):
            eng = nc.gpsimd if kc < 4 else nc.sync
            b_t = cbt_p.tile([P, L], F16, tag="b", name=f"b{kc}")
            eng.dma_start(b_t[:], bpp_d.ap()[kc * P:(kc + 1) * P, :])
            m_t = cbt_p.tile([P, L], F16, tag="m", name=f"m{kc}")
            eng.dma_start(m_t[:], mneg_d.ap()[kc * P:(kc + 1) * P, :])
            bm_tiles[kc] = (b_t, m_t)
        wo_t = []
        for p_ in range(4):
            t = wo_p.tile([P, D], F16, tag="wo", name=f"wo{p_}")
            nc.gpsimd.dma_start(t[:], wo_d.ap()[p_ * P:(p_ + 1) * P, :])
            wo_t.append(t)

        # ---- emission helpers ----
        qT, kT, v_aug = [None] * 4, [None] * 4, []
        ebm = [None] * 8

        def qk_proj(pc):
            for which, w_t, enc_t, out_list, bias, scl in (
                ("q", wq_t, eq, qT, wqb_sc, float(SCALE)),
                ("k", wk_t, ek, kT, wkb_c, 1.0),
            ):
                o = qkT_p.tile([P, L], F16, tag="qkT", name=f"{which}T{pc}")
                for sh in range(2):
                    ps = ps_s.tile([P, 512], F32, tag="ps_s",
                                   name=f"ps_{which}{pc}_{sh}")
                    for dc in range(4):
                        nc.tensor.matmul(
                            ps[:],
                            w_t[dc][:, pc * P:(pc + 1) * P],
                            enc_t[dc][:, sh * 512:(sh + 1) * 512],
                            start=(dc == 0), stop=(dc == 3))
                    sl = slice(sh * 512, (sh + 1) * 512)
                    nc.scalar.activation(o[:, sl], ps[:], AF.Identity,
                                         bias=bias[:, pc:pc + 1], scale=scl)
                out_list[pc] = o

        def v_proj():
            for sc in range(8):
                ps = ps_s.tile([P, 512], F32, tag="ps_s", name=f"ps_v{sc}")
                for dc in range(4):
                    nc.tensor.matmul(ps[:], ek[dc][:, sc * P:(sc + 1) * P],
                                     wv_t[dc][:], start=(dc == 0),
                                     stop=(dc == 3))
                va = v_p.tile([P, H * (HD + 1)], BF16, tag="v", name=f"v{sc}")
                vg = va[:].rearrange("p (h c) -> p h c", c=HD + 1)
                nc.vector.scalar_tensor_tensor(
                    vg[:, :, 0:HD],
                    ps[:].rearrange("p (h c) -> p h c", c=HD), 1.0,
                    wvb_bc[:].rearrange("p (h c) -> p h c", c=HD),
                    ALU.bypass, ALU.add)
                nc.vector.memset(vg[:, :, HD:HD + 1], 1.0)
                v_aug.append(va)

        def ebm_build(kc):
            b_t, m_t = bm_tiles[kc]
            cb_t = cbt_p.tile([P, L], F32, tag="cb", name=f"cb{kc}")
            nc.vector.scalar_tensor_tensor(
                cb_t[:], b_t[:], bw_col[:, 0:1], m_t[:], ALU.mult, ALU.add)
            e_t = ebm_p.tile([P, L], BF16, tag="ebm", name=f"ebm{kc}")
            nc.scalar.activation(e_t[:], cb_t[:], AF.Exp)
            ebm[kc] = e_t

        pending = []        # deferred eviction closures (emitted mid-next-head)

        def evictions(h, c_ps):
            # evict raw ctx (DVE -> fp16, partition-shifted for odd heads)
            # and the den row (ACT); reciprocal + DRAM-bounce broadcast for
            # the partition-wise 1/den, then normalize ctxn in place.
            # Deferred into the next head's stream so these ops never
            # head-of-line block the exp/mult queues at head boundaries.
            o = (h % 2) * HD
            pc = h // 2
            if h % 2 == 0:
                den_sb = den_p.tile([33, L], F32, tag="den", name=f"den{pc}")
                nc.vector.memset(den_sb[:], 1.0)
                evictions.den_sb = den_sb
            den_sb = evictions.den_sb
            r0 = (h % 2) * 32
            nc.scalar.copy(den_sb[r0:r0 + 1, :], c_ps[HD:HD + 1, :])
            nc.vector.tensor_copy(ctxn[pc][o:o + HD, :], c_ps[0:HD, :])
            if h % 2 == 1:
                rcp = den_p.tile([33, L], F32, tag="rcp", name=f"rcp{pc}")
                nc.vector.reciprocal_approx_fast(rcp[:], den_sb[:])
                for r_ in range(2):
                    nc.sync.dma_start(
                        den_d.ap()[2 * pc + r_:2 * pc + r_ + 1, :],
                        rcp[32 * r_:32 * r_ + 1, :])
                rb = rb_p.tile([P, L], F32, tag="rb", name=f"rb{pc}")
                nc.sync.dma_start(
                    rb[0:HD, :], bcast_ap(den_d.ap()[2 * pc:2 * pc + 1, :], HD))
                nc.sync.dma_start(
                    rb[HD:P, :], bcast_ap(den_d.ap()[2 * pc + 1:2 * pc + 2, :], HD))
                # normalize in place: ctxn *= 1/den
                nc.vector.tensor_mul(ctxn[pc][:], ctxn[pc][:], rb[:])

        def head(h):
            LAG = LAGS[0] if h == 0 else LAGS[1]
            o = (h % 2) * HD
            pc = h // 2
            # ctx accumulator: [65, 512] per qh, qh0 in free 0:512, qh1 in
            # 512:1024; den lands on partition 64 via the ones column.
            c_ps = ps_c.tile([P, L], F32, tag="ps_c", name=f"c_ps{h}")
            if h % 2 == 0:
                ctxn[pc] = ctxn_p.tile([P, L], F16, tag="ctxn",
                                       name=f"ctxn{pc}")
            e_ts = {}
            for kc in range(8 + LAG):
                if kc == 2:
                    while pending:
                        pending.pop(0)()
                if kc < 8:
                    s_ps = ps_s.tile([P, L], F32, tag="ps_s",
                                     name=f"s_ps_{h}_{kc}")
                    for qh in range(2):
                        sl = slice(qh * 512, (qh + 1) * 512)
                        nc.tensor.matmul(
                            s_ps[:, sl],
                            kT[pc][o:o + HD, kc * P:(kc + 1) * P],
                            qT[pc][o:o + HD, sl],
                            start=True, stop=True)
                    er = er_p.tile([P, L], BF16, tag="er", name=f"er_{h}_{kc}")
                    nc.scalar.activation(er[:], s_ps[:], AF.Exp)
                    et = e_p.tile([P, L], BF16, tag="e", name=f"e_{h}_{kc}")
                    eng = nc.gpsimd if kc % 3 == 2 else nc.vector
                    eng.tensor_mul(et[:], er[:], ebm[kc][:])
                    e_ts[kc] = et
                if kc >= LAG:
                    kp = kc - LAG
                    for qh in range(2):
                        sl = slice(qh * 512, (qh + 1) * 512)
                        nc.tensor.matmul(
                            c_ps[0:HD + 1, sl],
                            v_aug[kp][:, h * (HD + 1):(h + 1) * (HD + 1)],
                            e_ts[kp][:, sl],
                            start=(kp == 0), stop=(kp == 7))
            pending.append(lambda h=h, c=c_ps: evictions(h, c))

        ctxn = [None] * 4

        # ---- emission order ----
        # q/k projections feed pair pc = heads (2pc, 2pc+1); interleaving
        # them between pairs keeps the PE dense while ACT paces attention.
        qk_proj(0)
        ebm_build(0)
        ebm_build(1)
        v_proj()
        for kc in range(2, 8):
            ebm_build(kc)
        head(0)
        head(1)
        for pc in range(1, 4):
            qk_proj(pc)
            head(2 * pc)
            head(2 * pc + 1)
        while pending:
            pending.pop(0)()

        # ---- output projection ----
        # p-major emission: all pair-0 matmuls first, so the PE only waits
        # on the last pair's normalize chain for the final 8 matmuls.
        ps_c.release()
        ps_s.release()
        ps_o = tc.alloc_tile_pool(name="ps_o", bufs=8, space="PSUM")
        with tc.tile_pool(name="outp", bufs=3) as out_p:
            o_ps = [ps_o.tile([P, D], F32, tag="ps_o", name=f"o_ps{st}")
                    for st in range(8)]
            for p_ in range(4):
                for st in range(8):
                    nc.tensor.matmul(o_ps[st][:],
                                     ctxn[p_][:, st * P:(st + 1) * P],
                                     wo_t[p_][:],
                                     start=(p_ == 0), stop=(p_ == 3))
            for st in range(8):
                o_t = out_p.tile([P, D], F32, tag="out", name=f"out{st}")
                nc.vector.scalar_tensor_tensor(
                    o_t[:], o_ps[st][:], 1.0, wob_bc[:], ALU.bypass, ALU.add)
                nc.sync.dma_start(out_d.ap()[st * P:(st + 1) * P, :], o_t[:])
        ps_o.release()

    nc.compile()
    return nc


def shard_inputs(u_enc, e_enc, logit_bpp, ue_mask, eu_mask,
                 wq_k, wq_b, wk_k, wk_b, wv_k, wv_b, wo_k, wo_b,
                 bpp_w, bpp_b):
    """Build the 8 per-core input maps (layout + precision only).

    bpp_b is dropped: it shifts every logit in a row uniformly and
    cancels in softmax (as does the reference's +EPS).
    """
    u_enc = np.asarray(u_enc, np.float32)
    e_enc = np.asarray(e_enc, np.float32)
    bpp = np.asarray(logit_bpp, np.float32)
    ue_m = np.asarray(ue_mask).astype(np.float32)
    eu_m = np.asarray(eu_mask).astype(np.float32)

    def hf(x):
        return np.ascontiguousarray(x).astype(np.float16)

    com = dict(
        wq=hf(np.asarray(wq_k, np.float32).reshape(D, FH)),
        wk=hf(np.asarray(wk_k, np.float32).reshape(D, FH)),
        wv=hf(np.asarray(wv_k, np.float32).reshape(D, FH)),
        wo=hf(np.asarray(wo_k, np.float32).reshape(FH, D)),
        wqb=np.asarray(wq_b, np.float32).reshape(FH).copy(),
        wkb=np.asarray(wk_b, np.float32).reshape(FH).copy(),
        wvb=np.asarray(wv_b, np.float32).reshape(FH).copy(),
        wob=np.asarray(wo_b, np.float32).reshape(D).copy(),
        bppw=np.asarray(bpp_w, np.float32).reshape(1, 1).copy(),
    )
    uT = [hf(u_enc[b].T) for b in range(B)]
    eT = [hf(e_enc[b].T) for b in range(B)]
    bppT = hf(bpp.T)
    bppN = hf(bpp)
    # mask -> additive {0, -60000} encoding, [k, q] orientation
    ue_neg = [hf((ue_m[b, 0].T - 1.0) * 60000.0) for b in range(B)]
    eu_neg = [hf((eu_m[b, 0].T - 1.0) * 60000.0) for b in range(B)]
    in_maps = []
    for i in range(N_CORES):
        d, b = divmod(i, B)
        if d == 0:      # u queries, e keys -> u_update[b]
            m = dict(encQT=uT[b], encKT=eT[b], bpp=bppT, mneg=ue_neg[b])
        else:           # e queries, u keys -> e_update[b]
            m = dict(encQT=eT[b], encKT=uT[b], bpp=bppN, mneg=eu_neg[b])
        m.update(com)
        in_maps.append(m)
    return in_maps


_NC = None


def kernel(**inputs):
    global _NC
    if _NC is None:
        _NC = build_module()
    in_maps = shard_inputs(**inputs)
    res = bass_utils.run_bass_kernel_spmd(
        _NC, in_maps, core_ids=list(range(N_CORES)))
    u_update = np.stack([res.results[b]["out"] for b in range(B)])
    e_update = np.stack([res.results[B + b]["out"] for b in range(B)])
    return u_update, e_update


if __name__ == "__main__":
    # single-core CoreSim check of one (direction, batch) unit
    from concourse.bass_interp import CoreSim

    rng = np.random.default_rng(0)
    u = rng.standard_normal((B, L, D)).astype(np.float32)
    e = rng.standard_normal((B, L, D)).astype(np.float32)
    bpp = rng.standard_normal((L, L)).astype(np.float32)
    uem = (rng.random((B, 1, L, L)) < 0.9)
    eum = (rng.random((B, 1, L, L)) < 0.9)
    w = 1.0 / np.sqrt(D)
    wq = (rng.standard_normal((D, H, HD)) * w).astype(np.float32)
    wk = (rng.standard_normal((D, H, HD)) * w).astype(np.float32)
    wv = (rng.standard_normal((D, H, HD)) * w).astype(np.float32)
    wo = (rng.standard_normal((H, HD, D)) / np.sqrt(FH)).astype(np.float32)
    zq = (rng.standard_normal((H, HD)) * 0.1).astype(np.float32)
    zo = (rng.standard_normal((D,)) * 0.1).astype(np.float32)

    nc = build_module()
    in_maps = shard_inputs(u, e, bpp, uem, eum, wq, zq, wk, zq, wv, zq,
                           wo, zo, np.float32(1.3), np.float32(-0.2))

    core = int(__import__("os").environ.get("CORE", "0"))
    sim = CoreSim(nc, trace=False)
    for k, vv in in_maps[core].items():
        sim.tensor(k)[:] = vv
    sim.simulate(check_with_hw=False)
    got = np.array(sim.tensor("out"))

    def ref_unit(encQ, encK, bias_qk, mask_qk):
        q = SCALE * (encQ @ wq.reshape(D, FH) + zq.reshape(FH))
        kk = encK @ wk.reshape(D, FH) + zq.reshape(FH)
        vv = encK @ wv.reshape(D, FH) + zq.reshape(FH)
        accum = np.zeros((L, D), np.float64)
        for h in range(H):
            qi = q[:, h * HD:(h + 1) * HD]
            ki = kk[:, h * HD:(h + 1) * HD]
            vi = vv[:, h * HD:(h + 1) * HD]
            s = qi @ ki.T + bias_qk
            s = np.where(mask_qk, s, -np.inf)
            s = s - s.max(-1, keepdims=True)
            p_ = np.exp(s)
            p_ /= p_.sum(-1, keepdims=True)
            accum += (p_ @ vi) @ wo[h]
        return (accum + zo).astype(np.float32)

    bq = 1.3 * bpp + -0.2
    if core < B:
        exp_out = ref_unit(u[core], e[core], bq, uem[core, 0])
    else:
        exp_out = ref_unit(e[core - B], u[core - B], bq.T, eum[core - B, 0])
    err = np.abs(got - exp_out).max() / np.abs(exp_out).max()
    print("unit relerr vs numpy:", err)
